# revision 2
# baseline (speedup 1.0000x reference)
"""Trainium2 Bass kernel v7: position-split mixed precision (f16 + fp8 DR).

Building on v6 (537us): fp8e4m3 DoubleRow matmuls run 2x f16 MACs/instr on
TRN2. Pure fp8 everywhere fails the 2e-2 rel-err budget (e4m3 el-noise
~2.4% -> ~3-4% metric), but the error is driven by concentrated-attention
rows (small t, short windows). So precision is split by sequence position:

- projections: t <  512 f16, t >= 512 pure-fp8 DR pairs (x8, 32*w in e4m3)
- attention:   pair fp8 iff s-chunk >= 4 and tb >= 1 (q8/k8/v8/p8 casts)
- out-proj:    rows t < 1024 f16, t >= 1024 fp8 DR (enc8, 64*wo in e4m3)
- AllToAll row-interleave: dest core c gets rows {r: (r//64)%8 == c} so
  every core carries the same f16/fp8 row mix (balanced load); tc2=0 psum
  half is exactly the f16 rows, tc2=1 the fp8 rows.

numerics sim of this exact config: rel=1.56e-2 (budget 2e-2).
"""
import sys

if '/opt/trn_rl_repo' not in sys.path:
    sys.path.insert(0, '/opt/trn_rl_repo')

import numpy as np
import ml_dtypes

import concourse.bass as bass
import concourse.mybir as mybir
import concourse.tile as tile
from concourse import bacc
from concourse.bass_utils import run_bass_kernel_spmd

f32 = mybir.dt.float32
f16 = mybir.dt.float16
f8 = mybir.dt.float8e4
i32 = mybir.dt.int32
AF = mybir.ActivationFunctionType
Alu = mybir.AluOpType
DR = mybir.MatmulPerfMode.DoubleRow
E4 = ml_dtypes.float8_e4m3

N_CORES = 8
T, D, HD = 2048, 3584, 256
DC = D // 128
TWO_PI = 6.283185307179586
SOFT_CAP = 50.0
QUERY_SCALAR = 0.0625
WINDOW = 1024
MASK16 = 60000.0
SQ_C = QUERY_SCALAR * QUERY_SCALAR / 7500.0
EXP_BIAS = -4.0
W_SCALE = 32.0
WO_SCALE = 64.0

HC_PAIRS = ((0, 1, 'k'), (2, 3, 'v'), (4, 5, 'q0'), (6, 7, 'q1'))

# ---- custom DVE op: tn = (sq(in0)*s0 - 1)*in0 + in1 ------------------------
from concourse.dve_spec import Spec, Src0, Src1, C0, One, sq, lower as dve_lower
import concourse.dve_ops as dvo


def _register_softcap_op():
    name = "SOFTCAP_MASK_ANT"
    for op in dvo.OPS:
        if op.name == name:
            return op
    spec = Spec(
        body=(sq(Src0) * C0 - One) * Src0 + Src1,
        reference=lambda in0, in1, s0, s1, imm2:
            (in0.astype(np.float32) ** 2 * s0 - 1.0) * in0 + in1,
    )
    opcode = dvo._CUSTOM_DVE_ROW_BASE + len(dvo.OPS)
    assert opcode < 0x20
    shas = {}
    for ver in ("v3", "v4"):
        uops = dve_lower(spec, ver=ver)
        shas[ver] = dvo.DveOpSpec(name=name, opcode=opcode, uops=uops,
                                  rd1_en=True).sha(ver)
    op = dvo.DveOp(name, spec, subdim=False, uops_sha=shas)
    dvo.OPS.append(op)
    dvo._SUB_OPCODE_FOR_NAME[name] = opcode
    return op


SOFTCAP_OP = _register_softcap_op()


def _live_chunks(tb):
    t0 = tb * 512
    smin = max(0, t0 - (WINDOW - 1))
    smax = t0 + 511
    return list(range(smin // 128, smax // 128 + 1))


def _build_module():
    nc = bacc.Bacc("TRN2", target_bir_lowering=False, debug=False,
                   num_devices=N_CORES)

    x16_in = nc.declare_dram_parameter("x16", [D, 512], f16, isOutput=False)
    x8_in = nc.declare_dram_parameter("x8", [D, 1536], f8, isOutput=False)
    w16_in = nc.declare_dram_parameter("w16", [8, 128, D], f16,
                                       isOutput=False)
    w8_in = nc.declare_dram_parameter("w8", [8, 128, D], f8, isOutput=False)
    wo16_in = nc.declare_dram_parameter("wo16", [4096, D], f16,
                                        isOutput=False)
    wo8_in = nc.declare_dram_parameter("wo8", [16, 128, 2, D], f8,
                                       isOutput=False)
    posb_in = nc.declare_dram_parameter("posb", [128, T], f32, isOutput=False)
    cf_in = nc.declare_dram_parameter("cf", [128, 1], f32, isOutput=False)
    c16_in = nc.declare_dram_parameter("c16", [128, 128], f16, isOutput=False)
    out_ext = nc.declare_dram_parameter("out", [T // N_CORES, D], f16,
                                        isOutput=True)

    cc_in = [nc.dram_tensor(f"cc_in{h}", [8, 256, 256], f16) for h in range(2)]
    cc_out = [nc.dram_tensor(f"cc_out{h}", [8, 256, 256], f16)
              for h in range(2)]

    with tile.TileContext(nc) as tc:
        with tc.tile_pool(name="hold", bufs=1) as hold:
            ident16 = hold.tile([128, 128], f16)
            nc.sync.dma_start(ident16[:], c16_in[:, :])
            invts2 = hold.tile([128, 1], f32)
            nc.sync.dma_start(invts2[:], cf_in[:, 0:1])
            bias4 = hold.tile([128, 1], f32)
            nc.gpsimd.memset(bias4[:], EXP_BIAS)
            ones128 = hold.tile([128, 128], f16)
            nc.gpsimd.memset(ones128[:], 1.0)
            ones8 = hold.tile([128, 2, 128], f8)
            nc.gpsimd.memset(ones8[:], 1.0)

            pair_masks = {}
            for key, (dd0, dd1, cop) in {
                0: (0, 128, 'c'), 256: (256, 384, 'c'),
                -1024: (-1024, -896, 'w'), -768: (-768, -640, 'w'),
            }.items():
                m = hold.tile([128, 1024], f16, name=f"m{key}")
                nc.gpsimd.memset(m[:], 0.0)
                for ci, dd in enumerate((dd0, dd1)):
                    sl = m[:, ci * 512:(ci + 1) * 512]
                    if cop == 'c':
                        nc.gpsimd.affine_select(
                            out=sl, in_=sl, compare_op=Alu.is_ge,
                            fill=MASK16, base=-dd,
                            pattern=[[1, 512]], channel_multiplier=-1)
                    else:
                        nc.gpsimd.affine_select(
                            out=sl, in_=sl, compare_op=Alu.is_gt,
                            fill=MASK16, base=dd + WINDOW,
                            pattern=[[-1, 512]], channel_multiplier=1)
                pair_masks[key] = m
            zmask = hold.tile([128, 1024], f16, name="mz")
            nc.gpsimd.memset(zmask[:], 0.0)

            sin_t = hold.tile([128, T], f16)
            cos_t = hold.tile([128, T], f16)
            qT_t = [hold.tile([128, T], f16, name=f"qT{c}") for c in range(4)]
            kT_t = [hold.tile([128, T], f16, name=f"kT{c}") for c in range(2)]
            # fp8 copies of q/k for t,s >= 512 (hd-paired for DR)
            q8_t = [hold.tile([128, 2, 1536], f8, name=f"q8{lh}")
                    for lh in range(2)]
            k8_t = hold.tile([128, 2, 1536], f8, name="k8")
            # v: s-chunks 0..3 f16, 4..15 fp8 (s-chunk-paired for DR)
            v_t = [hold.tile([128, 256], f16, name=f"v{tc_}")
                   for tc_ in range(4)]
            vp8 = [hold.tile([128, 2, 256], f8, name=f"vp8_{pi}")
                   for pi in range(2, 8)]

            # ---------- phases 1+2 interleaved ----------
            ORDER = [(lh, src, ph) for lh in range(2) for src in range(8)
                     for ph in range(2)]
            ROUNDS = ((0, (0, 1, 2, 3)), (2048, (4, 5, 6)))
            wo16_meta = []
            wo8_meta = []
            for c0, dbs in ROUNDS:
                for i, (lh, src, ph) in enumerate(ORDER):
                    grow = (2 * src + lh) * 256 + ph * 128
                    wo16_meta.append((grow, c0, len(dbs)))
                for j in range(16):
                    wo8_meta.append((j, c0, len(dbs)))
            n_pref = 3
            n_pref8 = 2
            wo16_tiles = []
            wo8_tiles = []
            enc_tiles = {}

            with (
                tc.tile_pool(name="wop", bufs=3) as wop,
                tc.tile_pool(name="wop8", bufs=2) as wop8,
                tc.tile_pool(name="ge", bufs=1) as ge,
            ):
                def run_attention(lh, att, attb):
                    qA, qB = qT_t[2 * lh], qT_t[2 * lh + 1]
                    # 64-row-granule interleave: dest core cd gets cols
                    # [64*cd, 64*cd+64) of every tb block
                    esb_all = [attb.tile([128, 4, 512], f16,
                                         tag=f"esb{c}", name=f"esb{c}")
                               for c in range(2)]

                    def emit_tail(p):
                        (tb, sj0, sj1, ps_pair, is8, e0, e1, dbc,
                         first, last) = p
                        dd0 = sj0 * 128 - tb * 512
                        mk = pair_masks.get(dd0, zmask)
                        tn = att.tile([128, 1024], f16, tag="tn",
                                      name="tn")
                        nc.vector._custom_dve(SOFTCAP_OP, out=tn[:],
                                              in0=ps_pair[:], in1=mk[:],
                                              s0=SQ_C)
                        if is8:
                            pj = att.tile([128, 2, 512], f8, tag="pj8",
                                          name="pj8")
                            nc.scalar.activation(pj[:], tn[:], AF.Exp,
                                                 scale=-QUERY_SCALAR,
                                                 bias=bias4[:])
                            pi = sj0 // 2
                            nc.tensor.matmul(e0[:], vp8[pi - 2][:, :, 0:128],
                                             pj[:], start=first, stop=last,
                                             perf_mode=DR)
                            nc.tensor.matmul(e1[:],
                                             vp8[pi - 2][:, :, 128:256],
                                             pj[:], start=first, stop=last,
                                             perf_mode=DR)
                            nc.tensor.matmul(dbc[:], ones8[:], pj[:],
                                             start=first, stop=last,
                                             perf_mode=DR)
                        else:
                            pj = att.tile([128, 1024], f16, tag="pj",
                                          name="pj")
                            nc.scalar.activation(pj[:], tn[:], AF.Exp,
                                                 scale=-QUERY_SCALAR,
                                                 bias=bias4[:])
                            for ci, sj in enumerate((sj0, sj1)):
                                pjc = pj[:, ci * 512:(ci + 1) * 512]
                                f = first and ci == 0
                                l = last and ci == 1
                                nc.tensor.matmul(e0[:], v_t[sj][:, 0:128],
                                                 pjc, start=f, stop=l)
                                nc.tensor.matmul(e1[:], v_t[sj][:, 128:256],
                                                 pjc, start=f, stop=l)
                                nc.tensor.matmul(dbc[:], ones128[:], pjc,
                                                 start=f, stop=l)
                        if last:
                            rec = attb.tile([128, 512], f32, tag="rec",
                                            name="rec")
                            nc.vector.reciprocal_approx_fast(rec[:], dbc[:])
                            for c, e_ps in enumerate((e0, e1)):
                                nc.vector.tensor_tensor(
                                    esb_all[c][:, tb, :], e_ps[:],
                                    rec[:], Alu.mult)

                    with (
                        tc.tile_pool(name="ps_l", bufs=2,
                                     space="PSUM") as ps_lp,
                        tc.tile_pool(name="ps_e", bufs=1,
                                     space="PSUM") as ps_ep,
                        tc.tile_pool(name="ps_d", bufs=2,
                                     space="PSUM") as ps_dp,
                    ):
                        pend = None
                        for tb in range(4):
                            js = _live_chunks(tb)
                            npair = len(js) // 2
                            e0 = ps_ep.tile([128, 512], f32, tag="e0",
                                            name="e0")
                            e1 = ps_ep.tile([128, 512], f32, tag="e1",
                                            name="e1")
                            dbc = ps_dp.tile([128, 512], f32, tag="dbc",
                                             name="dbc")
                            qs = slice(tb * 512, (tb + 1) * 512)
                            qs8 = slice((tb - 1) * 512, tb * 512)
                            for pi_ in range(npair):
                                sj0, sj1 = js[2 * pi_], js[2 * pi_ + 1]
                                is8 = (sj0 >= 4 and tb >= 1)
                                ps_pair = ps_lp.tile([128, 1024], f32,
                                                     tag="l", name="l")
                                if is8:
                                    for ci, sj in enumerate((sj0, sj1)):
                                        ls = slice(ci * 512, (ci + 1) * 512)
                                        ks8 = slice(sj * 128 - 512,
                                                    sj * 128 - 512 + 128)
                                        nc.tensor.matmul(
                                            ps_pair[:, ls],
                                            k8_t[:, :, ks8],
                                            q8_t[lh][:, :, qs8],
                                            start=True, stop=True,
                                            perf_mode=DR)
                                else:
                                    for ci, sj in enumerate((sj0, sj1)):
                                        ks = slice(sj * 128, (sj + 1) * 128)
                                        ls = slice(ci * 512, (ci + 1) * 512)
                                        nc.tensor.matmul(ps_pair[:, ls],
                                                         kT_t[0][:, ks],
                                                         qA[:, qs],
                                                         start=True,
                                                         stop=False)
                                        nc.tensor.matmul(ps_pair[:, ls],
                                                         kT_t[1][:, ks],
                                                         qB[:, qs],
                                                         start=False,
                                                         stop=True)
                                if pend is not None:
                                    emit_tail(pend)
                                pend = (tb, sj0, sj1, ps_pair, is8, e0, e1,
                                        dbc, pi_ == 0, pi_ == npair - 1)
                        emit_tail(pend)

                    # batched interleave scatter: 16 big DMAs per lh
                    for c in range(2):
                        r0 = c * 128
                        for cd in range(8):
                            nc.sync.dma_start(
                                cc_in[lh][cd, r0:r0 + 128, 0:256],
                                esb_all[c][:, :, cd * 64:cd * 64 + 64])
                    nc.gpsimd.collective_compute(
                        "AllToAll", Alu.bypass,
                        replica_groups=[list(range(N_CORES))],
                        ins=[cc_in[lh][:]], outs=[cc_out[lh][:]])
                    for src in range(8):
                        for ph in range(2):
                            i = lh * 16 + src * 2 + ph
                            ec = ge.tile([128, 256], f16, tag=f"e{i}",
                                         name=f"e{i}")
                            nc.sync.dma_start(
                                ec[:],
                                cc_out[lh][src, ph * 128:(ph + 1) * 128, :])
                            enc_tiles[i] = ec

                # ---------- phase 0 + 1a ----------
                with (
                    tc.tile_pool(name="x8p", bufs=1) as x8p,
                    tc.tile_pool(name="w8p", bufs=2) as w8p,
                    tc.tile_pool(name="pp1", bufs=2) as pp1,
                  ):
                  with (
                    tc.tile_pool(name="x16p", bufs=1) as x16p,
                    tc.tile_pool(name="wfp", bufs=2) as wfp,
                    tc.tile_pool(name="p0", bufs=1) as p0,
                  ):
                    wf_list = {}
                    w8_list = {}
                    x16 = x16p.tile([128, 28, 512], f16, name="x16")
                    x8 = x8p.tile([128, 14, 2, 1536], f8, name="x8")
                    for hc in (0, 1):
                        wf = wfp.tile([128, D], f16, tag="wf",
                                      name=f"wf{hc}")
                        nc.sync.dma_start(wf[:], w16_in[hc, :, :])
                        w8t = w8p.tile([128, 14, 2, 128], f8, tag="w8",
                                       name=f"w8_{hc}")
                        nc.sync.dma_start(w8t[:], w8_in[hc, :, :])
                        wf_list[hc] = wf
                        w8_list[hc] = w8t
                        if hc == 0:
                            for d in range(4):
                                nc.sync.dma_start(
                                    x16[:, d, :],
                                    x16_in[d * 128:(d + 1) * 128, :])
                    for d in range(DC):
                        if d >= 4:
                            nc.sync.dma_start(
                                x16[:, d, :],
                                x16_in[d * 128:(d + 1) * 128, :])
                        nc.sync.dma_start(x8[:, d // 2, d % 2, :],
                                          x8_in[d * 128:(d + 1) * 128, :])

                    # phase 0: sin/cos tables (V/S only)
                    for hf in range(8):
                        hs = slice(hf * 256, (hf + 1) * 256)
                        posb = p0.tile([128, 256], f32, tag="p0p")
                        nc.sync.dma_start(posb[:], posb_in[:, hs])
                        for dst, shift in ((sin_t, 0.0), (cos_t, 0.25)):
                            a = p0.tile([128, 256], f32, tag="p0a")
                            nc.vector.tensor_scalar(a[:], posb[:], invts2[:],
                                                    shift, Alu.mult, Alu.add)
                            b = p0.tile([128, 256], i32, tag="p0b")
                            nc.vector.tensor_copy(b[:], a[:])
                            c = p0.tile([128, 256], f32, tag="p0c")
                            nc.vector.tensor_copy(c[:], b[:])
                            r = p0.tile([128, 256], f32, tag="p0r")
                            nc.vector.tensor_tensor(r[:], a[:], c[:],
                                                    Alu.subtract)
                            nc.scalar.activation(dst[:, hs], r[:], AF.Sin,
                                                 scale=TWO_PI)

                    vT_sb = [x16p.tile([128, T], f16, tag=f"vT{c}",
                                       name=f"vT{c}")
                             for c in range(2)]

                    def proj_pair(hcA, hcB, kind, ps_p,
                                  tqs=(0, 1, 2, 3)):
                        ps_pair = {}
                        for hc in (hcA, hcB):
                            if 0 in tqs:
                                if hc in wf_list:
                                    wf = wf_list[hc]
                                else:
                                    wf = wfp.tile([128, D], f16, tag="wf",
                                                  name=f"wf{hc}")
                                    nc.sync.dma_start(wf[:],
                                                      w16_in[hc, :, :])
                                    wf_list[hc] = wf
                            if len(tqs) > 1 or 0 not in tqs:
                                if hc in w8_list:
                                    w8t = w8_list[hc]
                                else:
                                    w8t = w8p.tile([128, 14, 2, 128], f8,
                                                   tag="w8",
                                                   name=f"w8_{hc}")
                                    nc.sync.dma_start(w8t[:],
                                                      w8_in[hc, :, :])
                                    w8_list[hc] = w8t
                            pss = {t: ps_p.tile([128, 512], f32,
                                                tag=f"ps{t}", name=f"ps{t}")
                                   for t in tqs}
                            ps_pair[hc] = pss
                            for t in tqs:
                                if t == 0:
                                    for d in range(DC):
                                        nc.tensor.matmul(
                                            pss[0][:],
                                            wf[:, d * 128:(d + 1) * 128],
                                            x16[:, d, :],
                                            start=(d == 0),
                                            stop=(d == DC - 1))
                                else:
                                    cs = slice((t - 1) * 512, t * 512)
                                    for j in range(14):
                                        nc.tensor.matmul(
                                            pss[t][:],
                                            w8t[:, j, :, :],
                                            x8[:, j, :, cs],
                                            start=(j == 0), stop=(j == 13),
                                            perf_mode=DR)
                        if kind == 'v':
                            for t in tqs:
                                ts_ = slice(t * 512, (t + 1) * 512)
                                if t == 0:
                                    nc.scalar.copy(vT_sb[0][:, ts_],
                                                   ps_pair[hcA][t][:])
                                    nc.scalar.copy(vT_sb[1][:, ts_],
                                                   ps_pair[hcB][t][:])
                                else:
                                    nc.scalar.activation(
                                        vT_sb[0][:, ts_], ps_pair[hcA][t][:],
                                        AF.Copy, scale=1.0 / W_SCALE)
                                    nc.scalar.activation(
                                        vT_sb[1][:, ts_], ps_pair[hcB][t][:],
                                        AF.Copy, scale=1.0 / W_SCALE)
                            return
                        if kind == 'k':
                            dstA, dstB = kT_t[0], kT_t[1]
                        elif kind == 'q0':
                            dstA, dstB = qT_t[0], qT_t[1]
                        else:
                            dstA, dstB = qT_t[2], qT_t[3]
                        for t in tqs:
                            ts_ = slice(t * 512, (t + 1) * 512)
                            sA = pp1.tile([128, 512], f16, tag="sA")
                            sB = pp1.tile([128, 512], f16, tag="sB")
                            if t == 0:
                                nc.scalar.copy(sA[:], ps_pair[hcA][t][:])
                                nc.scalar.copy(sB[:], ps_pair[hcB][t][:])
                            else:
                                nc.scalar.activation(
                                    sA[:], ps_pair[hcA][t][:], AF.Copy,
                                    scale=1.0 / W_SCALE)
                                nc.scalar.activation(
                                    sB[:], ps_pair[hcB][t][:], AF.Copy,
                                    scale=1.0 / W_SCALE)
                            cs_ = cos_t[:, ts_]
                            sn = sin_t[:, ts_]
                            t1 = pp1.tile([128, 512], f16, tag="t1")
                            t2 = pp1.tile([128, 512], f16, tag="t2")
                            nc.vector.tensor_tensor(t1[:], sA[:], cs_,
                                                    Alu.mult)
                            nc.vector.tensor_tensor(t2[:], sB[:], sn,
                                                    Alu.mult)
                            nc.vector.tensor_tensor(dstA[:, ts_], t1[:],
                                                    t2[:], Alu.subtract)
                            nc.vector.tensor_tensor(t1[:], sB[:], cs_,
                                                    Alu.mult)
                            nc.vector.tensor_tensor(t2[:], sA[:], sn,
                                                    Alu.mult)
                            nc.vector.tensor_tensor(dstB[:, ts_], t1[:],
                                                    t2[:], Alu.add)

                    with tc.tile_pool(name="ps_p", bufs=2,
                                      space="PSUM") as ps_p:
                        for hcA, hcB, kind in (HC_PAIRS[0], HC_PAIRS[1],
                                               HC_PAIRS[2]):
                            proj_pair(hcA, hcB, kind, ps_p)
                        proj_pair(6, 7, 'q1', ps_p, tqs=(0,))

                    # k8 + q8(lh0) casts (t >= 512)
                    nc.vector.tensor_copy(q8_t[0][:, 0, :], qT_t[0][:, 512:T])
                    nc.vector.tensor_copy(q8_t[0][:, 1, :], qT_t[1][:, 512:T])
                    nc.vector.tensor_copy(k8_t[:, 0, :], kT_t[0][:, 512:T])
                    nc.vector.tensor_copy(k8_t[:, 1, :], kT_t[1][:, 512:T])

                    with tc.tile_pool(name="ps_tr", bufs=4,
                                      space="PSUM") as ps_tr:
                        for tc_ in range(16):
                            for h2 in range(2):
                                tp = ps_tr.tile([128, 128], f16, tag="tr")
                                nc.tensor.transpose(
                                    tp[:],
                                    vT_sb[h2][:, tc_ * 128:(tc_ + 1) * 128],
                                    ident16[:])
                                if tc_ < 4:
                                    nc.vector.tensor_copy(
                                        v_t[tc_][:, h2 * 128:(h2 + 1) * 128],
                                        tp[:])
                                else:
                                    nc.vector.tensor_copy(
                                        vp8[tc_ // 2 - 2][
                                            :, tc_ % 2,
                                            h2 * 128:(h2 + 1) * 128],
                                        tp[:])

                    # issue q1 fp8 weight loads early (land before 1b)
                    for hc in (6, 7):
                        w8t = w8p.tile([128, 14, 2, 128], f8, tag="w8",
                                       name=f"w8_{hc}")
                        nc.sync.dma_start(w8t[:], w8_in[hc, :, :])
                        w8_list[hc] = w8t

                    # ---------- attention head 0 (overlaps A2A0 next) ------
                    with (
                        tc.tile_pool(name="att0", bufs=2) as att,
                        tc.tile_pool(name="attb0", bufs=2) as attb,
                    ):
                        run_attention(0, att, attb)

                    # wo prefetch starts under A2A0 flight
                    for idx in range(n_pref):
                        grow, c0, ndb = wo16_meta[idx]
                        wo_r = wop.tile([128, 2048], f16, tag="wo",
                                        name="wo")
                        nc.gpsimd.dma_start(wo_r[:, 0:ndb * 512],
                                            wo16_in[grow:grow + 128,
                                                    c0:c0 + ndb * 512])
                        wo16_tiles.append(wo_r)
                    for idx in range(n_pref8):
                        j, c0, ndb = wo8_meta[idx]
                        wo_r8 = wop8.tile([128, 2, 2048], f8, tag="wo8",
                                          name="wo8")
                        nc.scalar.dma_start(wo_r8[:, :, 0:ndb * 512],
                                            wo8_in[j, :, :,
                                                   c0:c0 + ndb * 512])
                        wo8_tiles.append(wo_r8)

                    # ---------- phase 1b: q1 projections under A2A0 -------
                    with tc.tile_pool(name="ps_p2", bufs=2,
                                      space="PSUM") as ps_p2:
                        proj_pair(6, 7, 'q1', ps_p2, tqs=(1, 2, 3))
                    nc.vector.tensor_copy(q8_t[1][:, 0, :], qT_t[2][:, 512:T])
                    nc.vector.tensor_copy(q8_t[1][:, 1, :], qT_t[3][:, 512:T])

                    # ---------- attention head 1 --------------------------
                    with (
                        tc.tile_pool(name="att1", bufs=2) as att,
                        tc.tile_pool(name="attb1", bufs=2) as attb,
                    ):
                        run_attention(1, att, attb)

                # enc8 casts for fp8 rows (cols 128:256 of each ec)
                enc8_tiles = {}
                with (
                    tc.tile_pool(name="wop", bufs=12) as wop,
                    tc.tile_pool(name="wop8", bufs=6) as wop8,
                    tc.tile_pool(name="e8", bufs=1) as e8p,
                ):
                    for j in range(16):
                        t8 = e8p.tile([128, 2, 128], f8, tag=f"t8_{j}",
                                      name=f"t8_{j}")
                        for i in range(2):
                            nc.vector.tensor_copy(
                                t8[:, i, :], enc_tiles[2 * j + i][:, 128:256])
                        enc8_tiles[j] = t8

                    # ---------- phase 3 ----------
                    with tc.tile_pool(name="o", bufs=4) as op_:
                        i16 = n_pref
                        i8 = n_pref8
                        for ri, (c0, dbs) in enumerate(ROUNDS):
                            with tc.tile_pool(name="ps_o", bufs=1,
                                              space="PSUM") as ps_op:
                                pso = {}
                                for tc2 in range(2):
                                    for db in dbs:
                                        pso[(tc2, db)] = ps_op.tile(
                                            [128, 512], f32,
                                            tag=f"o{tc2}_{db}",
                                            name=f"o{tc2}_{db}")
                                for j in range(16):
                                    # fp8 pair j (steps 2j, 2j+1), tc2=1
                                    gi8 = ri * 16 + j
                                    if gi8 < n_pref8:
                                        wo_r8 = wo8_tiles[gi8]
                                    else:
                                        pj_, cc0, ndb = wo8_meta[gi8]
                                        wo_r8 = wop8.tile([128, 2, 2048], f8,
                                                          tag="wo8",
                                                          name="wo8")
                                        eng8 = (nc.scalar if gi8 % 2 == 0
                                                else nc.sync)
                                        eng8.dma_start(
                                            wo_r8[:, :, 0:ndb * 512],
                                            wo8_in[pj_, :, :,
                                                   cc0:cc0 + ndb * 512])
                                    t8 = enc8_tiles[j]
                                    for kk, db in enumerate(dbs):
                                        nc.tensor.matmul(
                                            pso[(1, db)][:], t8[:],
                                            wo_r8[:, :,
                                                  kk * 512:(kk + 1) * 512],
                                            start=(j == 0), stop=(j == 15),
                                            perf_mode=DR)
                                    # f16 steps 2j, 2j+1, tc2=0
                                    for i in (2 * j, 2 * j + 1):
                                        gi = ri * 32 + i
                                        if gi < n_pref:
                                            wo_r = wo16_tiles[gi]
                                        else:
                                            grow, cc0, ndb = wo16_meta[gi]
                                            wo_r = wop.tile([128, 2048], f16,
                                                            tag="wo",
                                                            name="wo")
                                            eng = (nc.sync if gi % 2 == 0
                                                   else nc.scalar)
                                            eng.dma_start(
                                                wo_r[:, 0:ndb * 512],
                                                wo16_in[grow:grow + 128,
                                                        cc0:cc0 + ndb * 512])
                                        lh, src, ph = ORDER[i]
                                        ec = enc_tiles[lh * 16 + src * 2 + ph]
                                        st = ec[:, 0:128]
                                        for kk, db in enumerate(dbs):
                                            nc.tensor.matmul(
                                                pso[(0, db)][:], st,
                                                wo_r[:, kk * 512:
                                                     (kk + 1) * 512],
                                                start=(i == 0), stop=(i == 31))
                                for tc2 in range(2):
                                    for db in dbs:
                                        o_sb = op_.tile([128, 512], f16,
                                                        tag="o_sb")
                                        if tc2 == 0:
                                            nc.vector.tensor_copy(
                                                o_sb[:], pso[(tc2, db)][:])
                                        else:
                                            nc.scalar.activation(
                                                o_sb[:], pso[(tc2, db)][:],
                                                AF.Copy,
                                                scale=1.0 / WO_SCALE)
                                        nc.gpsimd.dma_start(
                                            out_ext[tc2 * 128:
                                                    (tc2 + 1) * 128,
                                                    db * 512:(db + 1) * 512],
                                            o_sb[:])

    nc.compile()
    return nc


_CACHE = {}
LAST_RESULTS = None


def _get_module():
    if "nc" not in _CACHE:
        _CACHE["nc"] = _build_module()
    return _CACHE["nc"]


def kernel(x, segment_pos, attn_mask, wq, wkv, wo):
    global LAST_RESULTS
    x = np.asarray(x, dtype=np.float32)
    segment_pos = np.asarray(segment_pos, dtype=np.int32)
    wq = np.asarray(wq, dtype=np.float32)
    wkv = np.asarray(wkv, dtype=np.float32)
    wo = np.asarray(wo, dtype=np.float32)

    nc = _get_module()

    xT = np.ascontiguousarray(x[0].T)                       # [D, T] f32
    x16 = np.ascontiguousarray(xT[:, 0:512].astype(np.float16))
    x8 = np.ascontiguousarray(xT[:, 512:T].astype(E4))
    posb = np.ascontiguousarray(
        np.broadcast_to(segment_pos[0].astype(np.float32)[None, :],
                        (128, T)))

    # wo16: rows grow = (2*src+lh)*256 + ph*128 == identity row layout
    wo16 = np.ascontiguousarray(wo.reshape(4096, D).astype(np.float16))
    # wo8: pair j = steps (2j, 2j+1); step s=(lh,src,ph):
    #   lh=s//16, src=(s%16)//2, ph=s%2; rows (2*src+lh)*256+ph*128 .. +128
    wo_flat = wo.reshape(4096, D)
    wo8 = np.empty((16, 128, 2, D), dtype=E4)
    for j in range(16):
        for i in range(2):
            s = 2 * j + i
            lh, src, ph = s // 16, (s % 16) // 2, s % 2
            grow = (2 * src + lh) * 256 + ph * 128
            wo8[j, :, i, :] = (wo_flat[grow:grow + 128, :]
                               * WO_SCALE).astype(E4)

    cf = ((10000.0 ** (-np.arange(128) / 128.0)) / TWO_PI).astype(
        np.float32).reshape(128, 1)
    c16 = np.eye(128, dtype=np.float16)

    def retile16(w_col):
        return np.ascontiguousarray(
            w_col.reshape(DC, 128, 128).transpose(1, 0, 2).reshape(128, D)
            .astype(np.float16))

    def retile8(w_col):
        return np.ascontiguousarray(
            (w_col * W_SCALE).reshape(DC, 128, 128).transpose(1, 0, 2)
            .reshape(128, D).astype(E4))

    in_maps = []
    for i in range(N_CORES):
        wk = wkv[0, i]
        wv = wkv[1, i]
        q0 = wq[2 * i]
        q1 = wq[2 * i + 1]
        cols = [wk[:, 0:128], wk[:, 128:256], wv[:, 0:128], wv[:, 128:256],
                q0[:, 0:128], q0[:, 128:256], q1[:, 0:128], q1[:, 128:256]]
        w16 = np.stack([retile16(c) for c in cols])
        w8 = np.stack([retile8(c) for c in cols])
        in_maps.append({
            "x16": x16,
            "x8": x8,
            "w16": np.ascontiguousarray(w16),
            "w8": np.ascontiguousarray(w8),
            "wo16": wo16,
            "wo8": wo8,
            "posb": posb,
            "cf": cf,
            "c16": c16,
        })

    LAST_RESULTS = run_bass_kernel_spmd(nc, in_maps,
                                        core_ids=list(range(N_CORES)))
    # row-interleave unshard: res[c].out[32*j + o] = full[256*j + 32*c + o]
    outs = [LAST_RESULTS.results[i]["out"] for i in range(N_CORES)]
    full = np.stack([o.reshape(4, 64, D) for o in outs], axis=1)
    out = full.reshape(T, D)
    return out[None, :, :].astype(np.float32)
